# revision 1
# baseline (speedup 1.0000x reference)
"""Trainium2 Bass kernel for the retrieval-KNN attention module.

Math (reference):
    qy     = y @ Wy_w.T + Wy_b              [B,L,D]
    kz     = dic_z @ Wz_w.T + Wz_b          [N,D]
    scores = (qy @ kz.T) / sqrt(D)          [B,L,N]
    attn   = softmax(scores, axis=-1)
    z      = (attn * prior) @ dic_z         [B,L,D]

Algebraic restructuring used here (exact up to float assoc.):
  * scores*sqrt(D) = qy @ (dic_z @ Wz_w.T).T = (qy @ Wz_w) @ dic_z.T
    so with ry := (y @ (Wy_w.T @ Wz_w) + Wy_b @ Wz_w) / sqrt(D),
    scores = ry @ dic_z.T.  The Wz_b term adds a per-row constant to
    scores, which softmax cancels exactly -> Wz_b drops out.
  * softmax needs no max-subtraction: scores here are O(1) (std ~0.3),
    exp() is safe in fp32.
  * prior folds into the exponent: prior*exp(s) = exp(s + ln(prior)).
  * the softmax denominator is obtained from the z-matmul itself by
    augmenting dic_z with an extra column 1/prior:
      sum_n exp(s+ln p)*(1/p) = sum_n exp(s) = den.
    That lands den[t] with t on partitions, exactly where the final
    per-partition normalization needs it.

Sharding: data-parallel over the 8192 tokens (B*L), 1024 tokens/core on
8 cores; dictionary/weights replicated. No cross-core communication.
Host does layout only (reshape/transpose/slice) - all arithmetic,
including the bias folds, ln/reciprocal of prior and all dtype
rounding, happens on device.
"""
import sys

sys.path.insert(0, "/opt/trn_rl_repo")

import numpy as np

B, L, D, N = 16, 512, 768, 8192
NCORES = 8
TOK = B * L                 # 8192 tokens total
T = TOK // NCORES           # 1024 tokens per core
DC = D // 128               # 6 chunks of the feature dim
NB = N // 128               # 64 dictionary blocks
GROUPS = [(0, 384), (384, 384), (768, 256)]  # token groups per core
SCALE = 1.0 / float(np.sqrt(np.float32(D)))

_cache = {}


def _build(debug=False, main_loop=True, byz=True):
    key = ("nc", debug, main_loop, byz)
    if key in _cache:
        return _cache[key]
    import concourse.mybir as mybir
    import concourse.tile as tile
    from concourse import bacc

    dt = mybir.dt
    f32, f32r, bf16 = dt.float32, dt.float32r, dt.bfloat16
    AF = mybir.ActivationFunctionType
    ALU = mybir.AluOpType

    nc = bacc.Bacc("TRN2", target_bir_lowering=False, debug=False,
                   num_devices=NCORES)

    # ---- DRAM I/O (per core) ----
    yT = nc.dram_tensor("yT", [D, T], f32r, kind="ExternalInput")
    wy = nc.dram_tensor("wy", [D, D], f32r, kind="ExternalInput")   # natural [e,d]
    wz = nc.dram_tensor("wz", [D, D], f32r, kind="ExternalInput")   # natural [e,d2]
    wyb = nc.dram_tensor("wyb", [D], f32, kind="ExternalInput")
    dzT = nc.dram_tensor("dzT", [D, N], f32, kind="ExternalInput")  # dic_z.T
    dz = nc.dram_tensor("dz", [N, D], f32r, kind="ExternalInput")   # dic_z
    prior = nc.dram_tensor("prior", [N], f32, kind="ExternalInput")
    zo = nc.dram_tensor("zo", [T, D], f32, kind="ExternalOutput")
    if debug:
        dbg_ry = nc.dram_tensor("dbg_ry", [DC, 128, T], f32, kind="ExternalOutput")
        dbg_w2 = nc.dram_tensor("dbg_w2", [DC, 128, D], f32, kind="ExternalOutput")
        dbg_pexp = nc.dram_tensor("dbg_pexp", [2, 128, 384], f32, kind="ExternalOutput")
        dbg_den = nc.dram_tensor("dbg_den", [3, 128, 4], f32, kind="ExternalOutput")


    with tile.TileContext(nc) as tc:
        # ---------- persistent SBUF ----------
        const = tc.alloc_tile_pool(name="const", bufs=1)
        dzt16 = [const.tile([128, N], bf16, name=f"dzt16_{c}") for c in range(DC)]
        ryt16 = [const.tile([128, T], bf16, name=f"ryt16_{c}") for c in range(DC)]
        pri_sb = const.tile([128, NB], f32, name="pri_sb")
        lnp_sb = const.tile([128, NB], f32, name="lnp_sb")
        rpri_sb = const.tile([128, NB], f32, name="rpri_sb")
        wyb_sb = const.tile([128, DC + 2], f32, name="wyb_sb")
        wyb_r = const.tile([128, DC + 2], f32r, name="wyb_r")
        byz_pb = const.tile([128, DC], f32, name="byz_pb")

        stream = tc.alloc_tile_pool(name="stream", bufs=8)
        work = tc.alloc_tile_pool(name="work", bufs=1)

        def cast_dzt_chunk(ch, cs=None):
            for c in (range(DC) if cs is None else cs):
                stg = stream.tile([128, 1024], f32, name="stg", tag="stg", bufs=3)
                nc.sync.dma_start(
                    out=stg[:],
                    in_=dzT.ap()[c * 128:(c + 1) * 128, ch * 1024:(ch + 1) * 1024])
                nc.vector.tensor_copy(
                    dzt16[c][:, ch * 1024:(ch + 1) * 1024], stg[:])

        with tc.tile_pool(name="s_outer", bufs=1) as s_outer:
            w2r = [s_outer.tile([128, D], f32r, name=f"w2r_{c}") for c in range(DC)]


            # ---- W2 = Wy.T @ Wz  (contract over e) ----
            with tc.tile_pool(name="s_inner", bufs=1) as s_inner, \
                 tc.tile_pool(name="pre_ps", space="PSUM", bufs=1) as pre_ps:
                wz_sb = [s_inner.tile([128, D], f32r, name=f"wz_sb_{c}") for c in range(DC)]
                for ec in range(DC):
                    nc.sync.dma_start(out=wz_sb[ec][:], in_=wz.ap()[ec * 128:(ec + 1) * 128, :])
                for dblk in range(DC):
                    pwA = pre_ps.tile([128, 512], f32, name="pwA", tag="pwA", bufs=2)
                    pwB = pre_ps.tile([128, 256], f32, name="pwB", tag="pwB", bufs=2)
                    for ec in range(DC):
                        wy_col = s_inner.tile([128, 128], f32r, name="wy_col",
                                              tag="wy_col", bufs=12)
                        nc.sync.dma_start(
                            out=wy_col[:],
                            in_=wy.ap()[ec * 128:(ec + 1) * 128,
                                        dblk * 128:(dblk + 1) * 128])
                        nc.tensor.matmul(pwA[:], wy_col[:], wz_sb[ec][:, 0:512],
                                         start=(ec == 0), stop=(ec == DC - 1))
                        nc.tensor.matmul(pwB[:], wy_col[:], wz_sb[ec][:, 512:768],
                                         start=(ec == 0), stop=(ec == DC - 1))
                    nc.vector.tensor_copy(w2r[dblk][:, 0:512], pwA[:])
                    nc.vector.tensor_copy(w2r[dblk][:, 512:768], pwB[:])

                nc.vector.memset(wyb_sb[:, DC:DC + 2], 0.0)
                nc.sync.dma_start(out=wyb_sb[:, 0:DC],
                                  in_=wyb.ap().rearrange("(c p) -> p c", p=128))
                nc.vector.tensor_copy(wyb_r[:], wyb_sb[:])

                # ---- byz = Wy_b @ Wz, computed as Wz.T @ Wy_b so the
                # result lands [d2-partition, 1] directly (byz_pb layout).
                # Moving operand is [128, 2] (fp32 moving size must be even;
                # col 1 is a zero pad whose output is ignored).
                if byz:
                    for d2blk in range(DC):
                        pbz = pre_ps.tile([128, 2], f32, name="pbz", tag="pwB",
                                          bufs=2)
                        for ec in range(DC):
                            nc.tensor.matmul(
                                pbz[:],
                                wz_sb[ec][:, d2blk * 128:(d2blk + 1) * 128],
                                wyb_r[:, ec:ec + 2],
                                start=(ec == 0), stop=(ec == DC - 1))
                        nc.vector.tensor_copy(byz_pb[:, d2blk:d2blk + 1], pbz[:, 0:1])
                nc.sync.dma_start(out=pri_sb[:],
                                  in_=prior.ap().rearrange("(b p) -> p b", p=128))
                nc.scalar.activation(lnp_sb[:], pri_sb[:], AF.Ln)
                nc.vector.reciprocal(rpri_sb[:], pri_sb[:])
            if not byz:
                nc.vector.memset(byz_pb[:], 0.0)

            # ---- ryT = (W2.T @ yT + byz) * SCALE, cast to bf16 ----
            with tc.tile_pool(name="s_yt", bufs=1) as s_yt, \
                 tc.tile_pool(name="ry_ps", space="PSUM", bufs=1) as ry_ps:
                for half in range(2):
                    h0 = half * 512
                    pry = [ry_ps.tile([128, 512], f32, name=f"pry{c}", tag=f"pry{c}")
                           for c in range(DC)]
                    for dc in range(DC):
                        yt_t = s_yt.tile([128, 512], f32r, name="yt_t", tag="yt_t",
                                         bufs=4)
                        nc.sync.dma_start(
                            out=yt_t[:],
                            in_=yT.ap()[dc * 128:(dc + 1) * 128, h0:h0 + 512])
                        for d2 in range(DC):
                            nc.tensor.matmul(
                                pry[d2][:],
                                w2r[dc][:, d2 * 128:(d2 + 1) * 128],
                                yt_t[:],
                                start=(dc == 0), stop=(dc == DC - 1))
                    for d2 in range(DC):
                        nc.vector.tensor_scalar(
                            out=ryt16[d2][:, h0:h0 + 512], in0=pry[d2][:],
                            scalar1=byz_pb[:, d2:d2 + 1], scalar2=SCALE,
                            op0=ALU.add, op1=ALU.mult)
                    cast_dzt_chunk(0, cs=range(3 * half, 3 * half + 3))




        if debug:
            for c in range(DC):
                st = work.tile([128, D], f32, name="dbgst", tag="dbgst", bufs=1)
                nc.vector.tensor_copy(st[:], w2r[c][:])
                nc.sync.dma_start(out=dbg_w2.ap()[c], in_=st[:])
            for c in range(DC):
                st = work.tile([128, T], f32, name="dbgst2", tag="dbgst2", bufs=1)
                nc.vector.tensor_copy(st[:], ryt16[c][:])
                nc.sync.dma_start(out=dbg_ry.ap()[c], in_=st[:])

        # ---------- main loop ----------
        with tc.tile_pool(name="main_ps", space="PSUM", bufs=1) as mps:
            for (g0, gsz) in (GROUPS if main_loop else []):
                ntt = gsz // 128
                pzA = [mps.tile([128, 512], f32, name=f"pzA{tt}", tag=f"pzA{tt}")
                       for tt in range(ntt)]
                pzB = [mps.tile([128, 258], f32, name=f"pzB{tt}", tag=f"pzB{tt}")
                       for tt in range(ntt)]
                for nb in range(NB):
                    if g0 == 0 and nb % 8 == 0 and 0 < nb:
                        cast_dzt_chunk(nb // 8)
                    # stream dic_z block + 1/prior column
                    dzs = stream.tile([128, D + 2], f32r, name="dzs", tag="dzs")
                    nc.sync.dma_start(out=dzs[:, 0:D],
                                      in_=dz.ap()[nb * 128:(nb + 1) * 128, :])
                    nc.vector.tensor_copy(
                        dzs[:, D:D + 2],
                        rpri_sb[:, nb:nb + 1].to_broadcast([128, 2]))

                    # scoresT[n-block, t-group]
                    ps_s = mps.tile([128, gsz], f32, name="ps_s", tag="ps_s", bufs=2)
                    for c in range(DC):
                        nc.tensor.matmul(
                            ps_s[:],
                            dzt16[c][:, nb * 128:(nb + 1) * 128],
                            ryt16[c][:, g0:g0 + gsz],
                            start=(c == 0), stop=(c == DC - 1))
                    # pexp = exp(scores + ln prior)  (prior folded in)
                    pexp = work.tile([128, gsz], f32r, name="pexp", tag="pexp", bufs=3)
                    nc.scalar.activation(pexp[:], ps_s[:], AF.Exp,
                                         bias=lnp_sb[:, nb:nb + 1])
                    if debug and g0 == 0 and nb < 2:
                        stp = work.tile([128, 384], f32, name="dbgp", tag="dbgp", bufs=2)
                        nc.vector.tensor_copy(stp[:], pexp[:])
                        nc.sync.dma_start(out=dbg_pexp.ap()[nb], in_=stp[:])
                    # z accumulation (+den in column 768)
                    for tt in range(ntt):
                        lhsT = pexp[:, tt * 128:(tt + 1) * 128]
                        nc.tensor.matmul(pzA[tt][:], lhsT, dzs[:, 0:512],
                                         start=(nb == 0), stop=(nb == NB - 1))
                        nc.tensor.matmul(pzB[tt][:], lhsT, dzs[:, 512:D + 2],
                                         start=(nb == 0), stop=(nb == NB - 1))
                # normalize + write out
                if debug:
                    for tt in range(ntt):
                        std = work.tile([128, 2], f32, name="dbgd", tag="dbgd", bufs=3)
                        nc.vector.tensor_copy(std[:], pzB[tt][:, 256:258])
                        nc.sync.dma_start(out=dbg_den.ap()[(g0 // 384)][:, tt:tt + 1],
                                          in_=std[:, 0:1])
                for tt in range(ntt):
                    rden = work.tile([128, 1], f32, name="rden", tag="rden", bufs=4)
                    nc.vector.reciprocal(rden[:], pzB[tt][:, 256:257])
                    z_sb = work.tile([128, D], f32, name="z_sb", tag="z_sb", bufs=3)
                    nc.vector.tensor_scalar_mul(z_sb[:, 0:512], pzA[tt][:], rden[:])
                    nc.vector.tensor_scalar_mul(z_sb[:, 512:768],
                                                pzB[tt][:, 0:256], rden[:])
                    r0 = g0 + tt * 128
                    nc.sync.dma_start(out=zo.ap()[r0:r0 + 128, :], in_=z_sb[:])

        work.release()
        stream.release()
        const.release()

    nc.compile()
    _cache[key] = nc
    if not debug and main_loop:
        _cache["nc"] = nc
    return nc


def kernel(y, Wy_w, Wy_b, Wz_w, Wz_b, dic_z, prior):
    # Wz_b is accepted but provably cancels (adds a per-row constant to the
    # pre-softmax scores); see module docstring.
    from concourse.bass_utils import run_bass_kernel_spmd

    nc = _build()

    y = np.asarray(y, dtype=np.float32)
    Wy_w = np.asarray(Wy_w, dtype=np.float32)
    Wy_b = np.asarray(Wy_b, dtype=np.float32)
    Wz_w = np.asarray(Wz_w, dtype=np.float32)
    dic_z = np.asarray(dic_z, dtype=np.float32)
    prior = np.asarray(prior, dtype=np.float32)

    yT_full = np.ascontiguousarray(y.reshape(TOK, D).T)          # [768, 8192]
    dzT_full = np.ascontiguousarray(dic_z.T)                     # [768, 8192]

    in_maps = []
    for c in range(NCORES):
        in_maps.append({
            "yT": np.ascontiguousarray(yT_full[:, c * T:(c + 1) * T]),
            "wy": Wy_w,
            "wz": Wz_w,
            "wyb": Wy_b,
            "dzT": dzT_full,
            "dz": dic_z,
            "prior": prior,
        })

    res = run_bass_kernel_spmd(nc, in_maps, list(range(NCORES)))
    out = np.concatenate([res.results[c]["zo"] for c in range(NCORES)], axis=0)
    return out.reshape(B, L, D).astype(np.float32)



# revision 10
# speedup vs baseline: 1.2433x; 1.2433x over previous
"""Trainium2 Bass kernel for the retrieval-KNN attention module.

Math (reference):
    qy     = y @ Wy_w.T + Wy_b              [B,L,D]
    kz     = dic_z @ Wz_w.T + Wz_b          [N,D]
    scores = (qy @ kz.T) / sqrt(D)          [B,L,N]
    attn   = softmax(scores, axis=-1)
    z      = (attn * prior) @ dic_z         [B,L,D]

Restructuring (exact up to float assoc.):
  * scores*sqrt(D) = (y @ (Wy_w.T @ Wz_w) + Wy_b @ Wz_w) @ dic_z.T
    =: ry @ dic_z.T.  Wz_b adds a per-row constant -> softmax drops it.
  * z = [sum_n w_n * (prior_n dic_z[n,:])] / [sum_n w_n],  w = exp(s).
    With w = 1 + dw, dw := exp(s)-1:
      z_num = c + sum_n dw_n * pdz[n,:],   c := sum_n prior_n dic_z[n,:]
      den   = N + sum_n dw_n
    c is accumulated in f32 on the tensor engine from the raw dic_z
    stream, which also exactly absorbs the column-mean of the fp8
    quantization error of pdz (the dominant error term otherwise).

fp8 usage (all rounding on device):
  * scores: DoubleRow fp8 matmuls. ry is split hi+lo (two fp8 casts,
    residual encoding ~ bf16 accuracy); dic_z.T is single fp8 (its
    quantization error is uncorrelated with dic_z and averages out).
  * z: dw (= exp(s)-1, quantized at its own ~3x smaller scale) and
    pdz = prior*dic_z, both single fp8, DoubleRow over n-block pairs.
    The den column is an exact fp8 1.0 appended to pdz.
  * Accumulations all in f32 PSUM; exp on the Act engine in f32.

Sharding: data-parallel over the 8192 tokens (B*L), 1024 tokens/core on
8 cores; dictionary/weights replicated. No cross-core communication.
Host does layout only (reshape/transpose/slice) - all arithmetic,
including fp8/bf16 rounding, happens on device.
"""
import sys

sys.path.insert(0, "/opt/trn_rl_repo")

import numpy as np

B, L, D, N = 16, 512, 768, 8192
NCORES = 8
TOK = B * L                 # 8192 tokens total
T = TOK // NCORES           # 1024 tokens per core
DC = D // 128               # 6 chunks of the feature dim
DP = DC // 2                # 3 chunk-pairs (DoubleRow)
NB = N // 128               # 64 dictionary blocks
NPAIR = NB // 2             # 32 block pairs (DoubleRow)
GROUPS = [(0, 256), (256, 384), (640, 384)]   # token groups per core
SCALE = 1.0 / float(np.sqrt(np.float32(D)))
FN = float(N)

_cache = {}


def _build(debug=False, main_loop=True):
    key = ("nc", debug, main_loop)
    if key in _cache:
        return _cache[key]
    import concourse.mybir as mybir
    import concourse.tile as tile
    from concourse import bacc

    dt = mybir.dt
    f32, f32r, fp8 = dt.float32, dt.float32r, dt.float8e4
    AF = mybir.ActivationFunctionType
    ALU = mybir.AluOpType
    DR = mybir.MatmulPerfMode.DoubleRow

    nc = bacc.Bacc("TRN2", target_bir_lowering=False, debug=False,
                   num_devices=NCORES)

    # ---- DRAM I/O (per core) ----
    yT = nc.dram_tensor("yT", [D, T], f32r, kind="ExternalInput")
    wy = nc.dram_tensor("wy", [D, D], f32r, kind="ExternalInput")   # natural [e,d]
    wz = nc.dram_tensor("wz", [D, D], f32r, kind="ExternalInput")   # natural [e,d2]
    wyb = nc.dram_tensor("wyb", [D], f32, kind="ExternalInput")
    dzT = nc.dram_tensor("dzT", [D, N], f32, kind="ExternalInput")  # dic_z.T
    dz = nc.dram_tensor("dz", [N, D], f32r, kind="ExternalInput")   # dic_z
    prior = nc.dram_tensor("prior", [N], f32, kind="ExternalInput")
    zo = nc.dram_tensor("zo", [T, D], f32, kind="ExternalOutput")
    if debug:
        dbg_ry = nc.dram_tensor("dbg_ry", [DC, 128, T], f32, kind="ExternalOutput")
        dbg_pdz = nc.dram_tensor("dbg_pdz", [2, 128, 770], f32, kind="ExternalOutput")
        dbg_c = nc.dram_tensor("dbg_c", [1, 770], f32, kind="ExternalOutput")
        dbg_dw = nc.dram_tensor("dbg_dw", [2, 128, 256], f32, kind="ExternalOutput")

    with tile.TileContext(nc) as tc:
        # ---------- persistent SBUF ----------
        const = tc.alloc_tile_pool(name="const", bufs=1)
        dzt8 = [const.tile([128, 2, N], fp8, name=f"dzt8_{c}") for c in range(DP)]
        ryh = [const.tile([128, 2, T], fp8, name=f"ryh_{c}") for c in range(DP)]
        ryl = [const.tile([128, 2, T], fp8, name=f"ryl_{c}") for c in range(DP)]
        pdz8 = [const.tile([128, 2, D + 2], fp8, name=f"pdz8_{p}")
                for p in range(NPAIR)]
        pri_sb = const.tile([128, NB], f32, name="pri_sb")
        pri_r = const.tile([128, NB], f32r, name="pri_r")
        wyb_sb = const.tile([128, DC + 2], f32, name="wyb_sb")
        wyb_r = const.tile([128, DC + 2], f32r, name="wyb_r")
        byz_pb = const.tile([128, DC], f32, name="byz_pb")
        c_sb = const.tile([1, D + 2], f32r, name="c_sb")
        one_sb = const.tile([1, 128], f32r, name="one_sb")
        one_f = const.tile([1, 128], f32, name="one_f")
        fn_f = const.tile([1, 2], f32, name="fn_f")

        stream = tc.alloc_tile_pool(name="stream", bufs=8)
        work = tc.alloc_tile_pool(name="work", bufs=1)

        nc.vector.memset(one_f[:], 1.0)
        nc.vector.tensor_copy(one_sb[:], one_f[:])
        nc.vector.memset(fn_f[:], FN)
        nc.vector.tensor_copy(c_sb[:, 768:770], fn_f[:])
        nc.sync.dma_start(out=pri_sb[:],
                          in_=prior.ap().rearrange("(b p) -> p b", p=128))
        nc.vector.tensor_copy(pri_r[:], pri_sb[:])

        def cast_dzt_chunk(ch, eng):
            # dzt8[c2//2][:, c2%2, ch*1024:(ch+1)*1024] <- fp8(dzT chunk)
            for c2 in range(DC):
                stg = stream.tile([128, 1024], f32, name="stg", tag="stg", bufs=2)
                nc.sync.dma_start(
                    out=stg[:],
                    in_=dzT.ap()[c2 * 128:(c2 + 1) * 128,
                                 ch * 1024:(ch + 1) * 1024])
                dst = dzt8[c2 // 2][:, c2 % 2, ch * 1024:(ch + 1) * 1024]
                if eng == "act":
                    nc.scalar.copy(dst, stg[:])
                elif eng == "pool":
                    nc.gpsimd.tensor_copy(dst, stg[:])
                else:
                    nc.vector.tensor_copy(dst, stg[:])

        with tc.tile_pool(name="s_outer", bufs=1) as s_outer:
            w2r = [s_outer.tile([128, D], f32r, name=f"w2r_{c}") for c in range(DC)]

            # ---- W2 = Wy.T @ Wz  (contract over e) ----
            with tc.tile_pool(name="s_inner", bufs=1) as s_inner, \
                 tc.tile_pool(name="pre_ps", space="PSUM", bufs=1) as pre_ps:
                wz_sb = [s_inner.tile([128, D], f32r, name=f"wz_sb_{c}")
                         for c in range(DC)]
                for ec in range(DC):
                    nc.sync.dma_start(out=wz_sb[ec][:],
                                      in_=wz.ap()[ec * 128:(ec + 1) * 128, :])
                for dblk in range(DC):
                    pwA = pre_ps.tile([128, 512], f32, name="pwA", tag="pwA", bufs=2)
                    pwB = pre_ps.tile([128, 256], f32, name="pwB", tag="pwB", bufs=2)
                    for ec in range(DC):
                        wy_col = s_inner.tile([128, 128], f32r, name="wy_col",
                                              tag="wy_col", bufs=6)
                        nc.sync.dma_start(
                            out=wy_col[:],
                            in_=wy.ap()[ec * 128:(ec + 1) * 128,
                                        dblk * 128:(dblk + 1) * 128])
                        nc.tensor.matmul(pwA[:], wy_col[:], wz_sb[ec][:, 0:512],
                                         start=(ec == 0), stop=(ec == DC - 1))
                        nc.tensor.matmul(pwB[:], wy_col[:], wz_sb[ec][:, 512:768],
                                         start=(ec == 0), stop=(ec == DC - 1))
                    nc.vector.tensor_copy(w2r[dblk][:, 0:512], pwA[:])
                    nc.vector.tensor_copy(w2r[dblk][:, 512:768], pwB[:])

                nc.vector.memset(wyb_sb[:, DC:DC + 2], 0.0)
                nc.sync.dma_start(out=wyb_sb[:, 0:DC],
                                  in_=wyb.ap().rearrange("(c p) -> p c", p=128))
                nc.vector.tensor_copy(wyb_r[:], wyb_sb[:])

                # byz = Wy_b @ Wz via Wz.T @ Wy_b -> [d2-partition, 1]
                for d2blk in range(DC):
                    pbz = pre_ps.tile([128, 2], f32, name="pbz", tag="pwB", bufs=2)
                    for ec in range(DC):
                        nc.tensor.matmul(
                            pbz[:],
                            wz_sb[ec][:, d2blk * 128:(d2blk + 1) * 128],
                            wyb_r[:, ec:ec + 2],
                            start=(ec == 0), stop=(ec == DC - 1))
                    nc.vector.tensor_copy(byz_pb[:, d2blk:d2blk + 1], pbz[:, 0:1])

            # ---- ryT = W2.T @ yT + byz (unscaled), split hi/lo fp8 ----
            with tc.tile_pool(name="ry_ps", space="PSUM", bufs=1) as ry_ps:
                for half in range(2):
                    h0 = half * 512
                    pry = [ry_ps.tile([128, 512], f32, name=f"pry{c}", tag=f"pry{c}")
                           for c in range(DC)]
                    for dc in range(DC):
                        yt_t = s_outer.tile([128, 512], f32r, name="yt_t",
                                            tag="yt_t", bufs=3)
                        nc.sync.dma_start(
                            out=yt_t[:],
                            in_=yT.ap()[dc * 128:(dc + 1) * 128, h0:h0 + 512])
                        for d2 in range(DC):
                            nc.tensor.matmul(
                                pry[d2][:],
                                w2r[dc][:, d2 * 128:(d2 + 1) * 128],
                                yt_t[:],
                                start=(dc == 0), stop=(dc == DC - 1))
                    for d2 in range(DC):
                        hi = ryh[d2 // 2][:, d2 % 2, h0:h0 + 512]
                        lo = ryl[d2 // 2][:, d2 % 2, h0:h0 + 512]
                        # hi = fp8(pry + byz) on Act; lo = (pry+byz)-hi on DVE
                        nc.scalar.activation(hi, pry[d2][:], AF.Identity,
                                             bias=byz_pb[:, d2:d2 + 1])
                        nc.vector.scalar_tensor_tensor(
                            out=lo, in0=pry[d2][:],
                            scalar=byz_pb[:, d2:d2 + 1],
                            in1=hi, op0=ALU.add, op1=ALU.subtract)

        if debug:
            for c in range(DC):
                st = work.tile([128, T], f32, name="dbgst2", tag="dbgst2", bufs=1)
                nc.vector.tensor_tensor(out=st[:], in0=ryh[c // 2][:, c % 2, :],
                                        in1=ryl[c // 2][:, c % 2, :], op=ALU.add)
                nc.sync.dma_start(out=dbg_ry.ap()[c], in_=st[:])

        # ---------- main loop ----------
        # g0 (256 tokens) also performs, per pair p:
        #   * dz pair DMA -> pdz8 cast (DVE) and exact-c accumulation (PE)
        #   * dzT chunk DMA -> dzt8 cast (chunk p//4, emitted one quantum early)
        for gi, (g0, gsz) in enumerate(GROUPS if main_loop else []):
            ntt = gsz // 128
            first = gi == 0
            with tc.tile_pool(name=f"main_ps{gi}", space="PSUM", bufs=1) as mps:
                if first:
                    cA = mps.tile([1, 512], f32, name="cA")
                    cB = mps.tile([1, 256], f32, name="cB")
                pzA = [mps.tile([128, 512], f32, name=f"pzA{tt}", tag=f"pzA{tt}")
                       for tt in range(ntt)]
                pzB = [mps.tile([128, 258], f32, name=f"pzB{tt}", tag=f"pzB{tt}")
                       for tt in range(ntt)]
                for p in range(NPAIR):
                    if first and p % 4 == 0:
                        cast_dzt_chunk(p // 4, "act" if p == 0 else "pool")
                        if p == 28:
                            pass
                    if first:
                        # stream dz rows [256, 768] pair-interleaved
                        dzp = stream.tile([128, 2, D], f32r, name="dzp",
                                          tag="dzp", bufs=4)
                        nc.sync.dma_start(
                            out=dzp[:],
                            in_=dz.ap()[p * 256:(p + 1) * 256, :].rearrange(
                                "(two p) d -> p two d", two=2))
                        nc.vector.memset(pdz8[p][:, :, D:D + 2], 1.0)
                        for i in range(2):
                            nb = 2 * p + i
                            nc.vector.tensor_scalar_mul(
                                pdz8[p][:, i, 0:D], dzp[:, i, :],
                                pri_sb[:, nb:nb + 1])
                            # exact c accumulation (f32)
                            nc.tensor.matmul(cA[:], pri_r[:, nb:nb + 1],
                                             dzp[:, i, 0:512],
                                             start=(nb == 0), stop=(nb == NB - 1))
                            nc.tensor.matmul(cB[:], pri_r[:, nb:nb + 1],
                                             dzp[:, i, 512:768],
                                             start=(nb == 0), stop=(nb == NB - 1))

                    dw8f = work.tile([128, 2, 384], fp8, name="dw8",
                                     tag="dw8", bufs=2)
                    dw8 = dw8f[:, :, 0:gsz]
                    for i in range(2):
                        nb = 2 * p + i
                        ps_s = mps.tile([128, gsz], f32, name="ps_s", tag="ps_s",
                                        bufs=2)
                        for k, rt in enumerate((ryh, ryl)):
                            for c in range(DP):
                                nc.tensor.matmul(
                                    ps_s[:],
                                    dzt8[c][:, :, nb * 128:(nb + 1) * 128],
                                    rt[c][:, :, g0:g0 + gsz],
                                    start=(k == 0 and c == 0),
                                    stop=(k == 1 and c == DP - 1),
                                    perf_mode=DR)
                        # w = exp(s*SCALE); dw = w - 1 in fp8
                        pex = work.tile([128, gsz], f32, name="pex", tag="pex",
                                        bufs=3)
                        nc.scalar.activation(pex[:], ps_s[:], AF.Exp, scale=SCALE)
                        nc.vector.tensor_scalar_add(dw8[:, i, :], pex[:], -1.0)
                    for tt in range(ntt):
                        lhsT = dw8[:, :, tt * 128:(tt + 1) * 128]
                        nc.tensor.matmul(pzA[tt][:], lhsT, pdz8[p][:, :, 0:512],
                                         start=(p == 0), stop=False, perf_mode=DR)
                        nc.tensor.matmul(pzB[tt][:], lhsT, pdz8[p][:, :, 512:D + 2],
                                         start=(p == 0), stop=False, perf_mode=DR)
                if first:
                    # c_sb = [c (exact), N, N]  (cols 768:770 preset to N)
                    nc.vector.tensor_copy(c_sb[:, 0:512], cA[:])
                    nc.vector.tensor_copy(c_sb[:, 512:768], cB[:])
                    if debug:
                        stc = work.tile([1, 770], f32, name="dbgc", tag="dbgc",
                                        bufs=1)
                        nc.vector.tensor_copy(stc[:], c_sb[:])
                        nc.sync.dma_start(out=dbg_c.ap()[:], in_=stc[:])
                # inject c into z_num (and N into den); closes the accum group
                for tt in range(ntt):
                    nc.tensor.matmul(pzA[tt][:], one_sb[:], c_sb[:, 0:512],
                                     start=False, stop=True)
                    nc.tensor.matmul(pzB[tt][:], one_sb[:], c_sb[:, 512:D + 2],
                                     start=False, stop=True)
                # normalize + write out
                for tt in range(ntt):
                    rden = work.tile([128, 1], f32, name="rden", tag="rden", bufs=4)
                    nc.vector.reciprocal(rden[:], pzB[tt][:, 256:257])
                    z_sb = work.tile([128, D], f32, name="z_sb", tag="z_sb", bufs=3)
                    nc.vector.tensor_scalar_mul(z_sb[:, 0:512], pzA[tt][:], rden[:])
                    nc.vector.tensor_scalar_mul(z_sb[:, 512:768],
                                                pzB[tt][:, 0:256], rden[:])
                    r0 = g0 + tt * 128
                    nc.sync.dma_start(out=zo.ap()[r0:r0 + 128, :], in_=z_sb[:])

        if debug:
            for j in range(2):
                st = work.tile([128, 770], f32, name="dbgpdz", tag="dbgpdz", bufs=2)
                nc.vector.tensor_copy(st[:], pdz8[0][:, j, :])
                nc.sync.dma_start(out=dbg_pdz.ap()[j], in_=st[:])

        work.release()
        stream.release()
        const.release()

    nc.compile()
    _cache[key] = nc
    if not debug and main_loop:
        _cache["nc"] = nc
    return nc


def kernel(y, Wy_w, Wy_b, Wz_w, Wz_b, dic_z, prior):
    # Wz_b is accepted but provably cancels (adds a per-row constant to the
    # pre-softmax scores); see module docstring.
    from concourse.bass_utils import run_bass_kernel_spmd

    nc = _build()

    y = np.asarray(y, dtype=np.float32)
    Wy_w = np.asarray(Wy_w, dtype=np.float32)
    Wy_b = np.asarray(Wy_b, dtype=np.float32)
    Wz_w = np.asarray(Wz_w, dtype=np.float32)
    dic_z = np.asarray(dic_z, dtype=np.float32)
    prior = np.asarray(prior, dtype=np.float32)

    yT_full = np.ascontiguousarray(y.reshape(TOK, D).T)          # [768, 8192]
    dzT_full = np.ascontiguousarray(dic_z.T)                     # [768, 8192]

    in_maps = []
    for c in range(NCORES):
        in_maps.append({
            "yT": np.ascontiguousarray(yT_full[:, c * T:(c + 1) * T]),
            "wy": Wy_w,
            "wz": Wz_w,
            "wyb": Wy_b,
            "dzT": dzT_full,
            "dz": dic_z,
            "prior": prior,
        })

    res = run_bass_kernel_spmd(nc, in_maps, list(range(NCORES)))
    out = np.concatenate([res.results[c]["zo"] for c in range(NCORES)], axis=0)
    return out.reshape(B, L, D).astype(np.float32)
